# revision 17
# baseline (speedup 1.0000x reference)
"""BilinearDiscriminator: sigmoid((x @ W.T) @ y.T) on 8 TRN2 NeuronCores.

Sharding: rows of x (= rows of the [16384, 16384] output) are split across
the 8 cores; y and W are replicated. Each core computes a [2048, 16384]
fp32 output shard independently (no collectives) and the host concatenates.

Per-core kernel:
  1. DMA in W [128,128], x_shard [2048,128], y [16384,128].
  2. PE-transpose W, x, y into d-major (contraction-on-partition) layout.
  3. xtT = W @ x.T            ([128, 2048], 4 matmuls)
  4. scores tile [128, 512] = xtT_tile.T @ yT_slice  (512 matmuls)
  5. sigmoid on ScalarE, PSUM -> SBUF
  6. batched 4 MB DMA writes of the output shard (memory-bound roofline).
"""

import os

import numpy as np

import concourse.bass as bass
import concourse.mybir as mybir
import concourse.tile as tile
from concourse import bacc
from concourse.bass_utils import run_bass_kernel_spmd
from concourse.masks import make_identity

N_CORES = 8
N, M, D = 16384, 16384, 128
N_LOC = N // N_CORES  # 2048 output rows per core

F32 = mybir.dt.float32
F32R = mybir.dt.float32r

# Big-GEMM input dtype: float32r streams 1 row/cycle (vs 4 for float32) at
# free-dim >= 256. Flip with BASS_BILIN_F32R=0 to fall back to full fp32.
USE_F32R = os.environ.get("BASS_BILIN_F32R", "1") == "1"

# Tuned on HW (per-iteration time, 8-core SPMD):
#   OUT_CHUNK=8192/bufs=2: ~457 us; 4096/3: ~446 us; 2048/6: ~434 us.
# Small chunks smooth the ACT->DMA pipeline (no m-tile-boundary bubbles).
OUT_CHUNK = int(os.environ.get("BILIN_OUT_CHUNK", "2048"))
OUT_BUFS = int(os.environ.get("BILIN_OUT_BUFS", "6"))
Y_CHUNK_TILES = int(os.environ.get("BILIN_Y_CHUNK_TILES", "32"))
Y_BUFS = int(os.environ.get("BILIN_Y_BUFS", "2"))
VAR_CHUNKS = os.environ.get("BILIN_VAR_CHUNKS", "0") == "1"


def _split_chunks_first(out_chunk: int) -> list[int]:
    """Column-chunk widths for the first m-tile: small leading chunks so the
    output-DMA stream starts early, then full-size chunks. Sums to M."""
    lead = [c for c in (2048, 2048, 4096) if c < out_chunk]
    rest = M - sum(lead)
    return lead + [out_chunk] * (rest // out_chunk) + (
        [rest % out_chunk] if rest % out_chunk else []
    )


def _build_nc(use_f32r: bool, reps: int = 1, timing_mode: bool = False):
    """reps > 1 re-executes the whole body in a For_i loop.

    timing_mode=True makes x/y/W/out *internal* DRAM scratch (contents
    irrelevant) with only a tiny dummy external input/output, so a call
    transfers ~nothing through the axon tunnel and wall-clock differences
    between two rep counts isolate pure device time."""
    from contextlib import nullcontext

    nc = bacc.Bacc("TRN2", target_bir_lowering=False, debug=False)

    if timing_mode:
        x_d = nc.dram_tensor("x", [N_LOC, D], F32)
        y_d = nc.dram_tensor("y", [M, D], F32)
        w_d = nc.dram_tensor("W", [D, D], F32)
        out_d = nc.dram_tensor("out", [N_LOC, M], F32)
        dummy_out = nc.dram_tensor("dummy_out", [1, 64], F32, kind="ExternalOutput")
    else:
        x_d = nc.dram_tensor("x", [N_LOC, D], F32, kind="ExternalInput")
        y_d = nc.dram_tensor("y", [M, D], F32, kind="ExternalInput")
        w_d = nc.dram_tensor("W", [D, D], F32, kind="ExternalInput")
        out_d = nc.dram_tensor("out", [N_LOC, M], F32, kind="ExternalOutput")

    XT_TILES = N_LOC // 128  # 16
    Y_CHUNKS = M // (Y_CHUNK_TILES * 128)

    with tile.TileContext(nc) as tc:
        with (
            tc.tile_pool(name="const", bufs=1) as cpool,
            tc.tile_pool(name="yraw", bufs=Y_BUFS) as ypool,
            tc.tile_pool(name="big", bufs=1) as bpool,
            tc.tile_pool(name="outp", bufs=OUT_BUFS) as opool,
            tc.tile_pool(name="tpsum", bufs=2, space="PSUM") as tpsum,
            tc.tile_pool(name="mpsum", bufs=6, space="PSUM") as mpsum,
        ):
            identity = cpool.tile([128, 128], F32)
            make_identity(nc, identity)

            # --- W -> wT (partition = d_in) ---
            w_sb = cpool.tile([128, 128], F32)
            nc.sync.dma_start(out=w_sb[:], in_=w_d[:, :])
            wt_ps = tpsum.tile([128, 128], F32, tag="tps")
            nc.tensor.transpose(wt_ps[:], w_sb[:], identity[:])
            w_t = cpool.tile([128, 128], F32)
            nc.vector.tensor_copy(w_t[:], wt_ps[:])

            loop_cm = tc.For_i(0, reps, 1) if reps > 1 else nullcontext()
            with loop_cm:
                _emit_body(
                    nc, tc, use_f32r, x_d, y_d, out_d,
                    identity, w_t, ypool, bpool, opool, tpsum, mpsum,
                    XT_TILES, Y_CHUNK_TILES, Y_CHUNKS,
                )

            if timing_mode:
                # Read back a sliver of `out` so its writers aren't dead code.
                nc.sync.dma_start(out=dummy_out[:, :], in_=out_d[0:1, 0:64])

    nc.compile()
    return nc


def _emit_body(
    nc, tc, use_f32r, x_d, y_d, out_d,
    identity, w_t, ypool, bpool, opool, tpsum, mpsum,
    XT_TILES, Y_CHUNK_TILES, Y_CHUNKS,
):
    if True:  # keep indentation close to original structure
        if True:
            mm_dt = F32R if use_f32r else F32

            # --- x -> xT (partition = d) ---
            # x viewed as [128, 16, 128]: partition = row-within-tile.
            x_view = x_d.rearrange("(t p) d -> p t d", p=128)
            x_sb = bpool.tile([128, XT_TILES, 128], F32)
            nc.sync.dma_start(out=x_sb[:], in_=x_view)
            xT = bpool.tile([128, N_LOC], F32)
            for t in range(XT_TILES):
                xt_ps = tpsum.tile([128, 128], F32, tag="tps")
                nc.tensor.transpose(xt_ps[:], x_sb[:, t, :], identity[:])
                nc.vector.tensor_copy(xT[:, t * 128 : (t + 1) * 128], xt_ps[:])

            # --- xtT = W @ x.T  [d_out=128, N_LOC] ---
            # For the f32r big GEMM the verifier requires operands to be
            # produced as float32r (rounded) — cast during the PSUM copy.
            xtT = bpool.tile([128, N_LOC], mm_dt)
            for c in range(N_LOC // 512):
                mm_ps = mpsum.tile([128, 512], F32, tag="mps")
                nc.tensor.matmul(
                    mm_ps[:],
                    w_t[:],
                    xT[:, c * 512 : (c + 1) * 512],
                    start=True,
                    stop=True,
                )
                nc.vector.tensor_copy(xtT[:, c * 512 : (c + 1) * 512], mm_ps[:])

            # --- y -> yT (partition = d), chunked loads ---
            y_view = y_d.rearrange("(t p) d -> p t d", p=128)  # [128, 128, 128]
            yT = bpool.tile([128, M], mm_dt)
            for c in range(Y_CHUNKS):
                y_sb = ypool.tile([128, Y_CHUNK_TILES, 128], F32, tag="ych")
                nc.sync.dma_start(
                    out=y_sb[:],
                    in_=y_view[:, c * Y_CHUNK_TILES : (c + 1) * Y_CHUNK_TILES, :],
                )
                for k in range(Y_CHUNK_TILES):
                    t = c * Y_CHUNK_TILES + k
                    yt_ps = tpsum.tile([128, 128], F32, tag="tps")
                    nc.tensor.transpose(yt_ps[:], y_sb[:, k, :], identity[:])
                    nc.vector.tensor_copy(yT[:, t * 128 : (t + 1) * 128], yt_ps[:])

            # --- main GEMM + sigmoid + output DMA ---
            # Small chunks on the first/last row-tile start the output-DMA
            # stream earlier and shrink the drain tail; 8192-wide (4 MB)
            # chunks elsewhere for DMA efficiency.
            for i in range(XT_TILES):
                lhsT = xtT[:, i * 128 : (i + 1) * 128]
                if VAR_CHUNKS and i == 0:
                    chunks = _split_chunks_first(OUT_CHUNK)
                elif VAR_CHUNKS and i == XT_TILES - 1:
                    chunks = _split_chunks_first(OUT_CHUNK)[::-1]
                else:
                    chunks = [OUT_CHUNK] * (M // OUT_CHUNK)
                col0 = 0
                for w in chunks:
                    out_sb = opool.tile([128, OUT_CHUNK], F32, tag="outc")
                    for j in range(w // 512):
                        col = col0 + j * 512
                        rhs = yT[:, col : col + 512]
                        mm = mpsum.tile([128, 512], F32, tag="mps")
                        nc.tensor.matmul(mm[:], lhsT, rhs, start=True, stop=True)
                        nc.scalar.activation(
                            out_sb[:, j * 512 : (j + 1) * 512],
                            mm[:],
                            mybir.ActivationFunctionType.Sigmoid,
                        )
                    nc.sync.dma_start(
                        out=out_d[i * 128 : (i + 1) * 128, col0 : col0 + w],
                        in_=out_sb[:, 0:w],
                    )
                    col0 += w


_NC_CACHE: dict = {}


def _get_nc(use_f32r: bool):
    key = use_f32r
    if key not in _NC_CACHE:
        _NC_CACHE[key] = _build_nc(use_f32r)
    return _NC_CACHE[key]


def run(x, y, W, trace: bool = False, use_f32r: bool = USE_F32R):
    """Run on 8 cores; returns (out [N, M] fp32, BassKernelResults)."""
    x = np.ascontiguousarray(np.asarray(x), dtype=np.float32)
    y = np.ascontiguousarray(np.asarray(y), dtype=np.float32)
    W = np.ascontiguousarray(np.asarray(W), dtype=np.float32)
    assert x.shape == (N, D) and y.shape == (M, D) and W.shape == (D, D)

    nc = _get_nc(use_f32r)
    in_maps = [
        {"x": x[c * N_LOC : (c + 1) * N_LOC], "y": y, "W": W}
        for c in range(N_CORES)
    ]
    try:
        res = run_bass_kernel_spmd(
            nc, in_maps, core_ids=list(range(N_CORES)), trace=trace
        )
    except ModuleNotFoundError:
        # This axon build has no NTFF profile hook (antenv.axon_hooks) —
        # retry with tracing hard-disabled.
        os.environ["BASS_NEVER_TRACE"] = "1"
        res = run_bass_kernel_spmd(
            nc, in_maps, core_ids=list(range(N_CORES)), trace=False
        )
    out = np.concatenate([res.results[c]["out"] for c in range(N_CORES)], axis=0)
    return out, res


def kernel(x, y, W):
    out, _ = run(x, y, W)
    return out
